# revision 3
# baseline (speedup 1.0000x reference)
"""Trainium2 Bass kernel for per-node multi-head attention.

Computation (per node n, fully independent across nodes):
    Q = h @ Wq.T  viewed (nh, hd)        [row-major reshape]
    K = h @ Wk.T  viewed (hd, nh)
    V = h @ Wv.T  viewed (hd, nh)
    comp[hh, g] = sum_d Q[hh, d] K[d, g] / 128
    scores = softmax(comp, axis=-1)
    out[l, d]  = sum_g scores[l, g] V[d, g]
    final = flat(out.T) @ Wfc.T

Sharding: data-parallel over the node dim N across 8 NeuronCores; weights
replicated; no collectives.

Per-core mapping:
  - TensorE: transpose h blocks (128x128), the 3 projections + final FC as
    float32r matmuls (full-rate fp32), transpose of the attention output.
  - VectorE: batched per-node einsums as broadcast-AP elementwise products
    (bf16) + segmented tensor_reduce, plus softmax normalization.
  - ScalarE: exp, PSUM->SBUF copies (with dtype casts).
  - Wk's rows are permuted during on-device weight prep so the K projection
    lands g-major (f' = g*64 + d), which makes the comp product APs unit-
    stride in the innermost dim.
"""

import numpy as np

N_FULL = 65536
H = 1024
NCORES = 8
NPC = N_FULL // NCORES  # rows per core
NH = 16                 # heads
HD = 64                 # head dim
KT = H // 128           # k tiles per contraction (8)

_BUILD_CACHE = {}


def _build(n_rows):
    if n_rows in _BUILD_CACHE:
        return _BUILD_CACHE[n_rows]

    import concourse.bass as bass
    import concourse.mybir as mybir
    import concourse.tile as tile
    from concourse import bacc
    from concourse.masks import make_identity

    f32 = mybir.dt.float32
    f32r = mybir.dt.float32r
    bf16 = mybir.dt.bfloat16
    MULT = mybir.AluOpType.mult
    ADD = mybir.AluOpType.add
    AXX = mybir.AxisListType.X

    nc = bacc.Bacc("TRN2", target_bir_lowering=False, debug=False)

    h_d = nc.dram_tensor("h", [n_rows, H], f32, kind="ExternalInput").ap()
    w_d = {
        name: nc.dram_tensor(name, [H, H], f32, kind="ExternalInput").ap()
        for name in ("wq", "wk", "wv", "wfc")
    }
    out_d = nc.dram_tensor("out", [n_rows, H], f32, kind="ExternalOutput").ap()

    ntiles = n_rows // 128

    def ap(base, offset_elems, dims):
        """Manual AP: dims are [step, count] FREE dims; partition from base."""
        b = base if isinstance(base, bass.AP) else base[...]
        return bass.AP(
            tensor=b.tensor,
            offset=b.offset + offset_elems,
            ap=[list(b.ap[0])] + [list(d) for d in dims],
        )

    with tile.TileContext(nc) as tc:
        with tc.tile_pool(name="const", bufs=1) as const_pool:
            ident = const_pool.tile([128, 128], f32)
            make_identity(nc, ident)

            # Transposed weights, SBUF-resident for the whole kernel.
            # wt[p, kt, f] = W[f, kt*128 + p]   (for wk: f is permuted g-major)
            wts = {}
            with tc.tile_pool(name="wprep", bufs=2) as wnat_pool, \
                 tc.tile_pool(name="wtps", bufs=4, space="PSUM") as wt_psum:
                for name in ("wq", "wk", "wv", "wfc"):
                    wt = const_pool.tile([128, KT, H], f32r, tag=f"wt_{name}")
                    wts[name] = wt
                    wnat = wnat_pool.tile([128, KT, H], f32, tag="wnat")
                    nc.sync.dma_start(
                        out=wnat,
                        in_=w_d[name].rearrange("(ft p) c -> p ft c", p=128),
                    )
                    for ft in range(KT):
                        for kt in range(KT):
                            ps = wt_psum.tile([128, 128], f32, tag="wt_ps")
                            nc.tensor.transpose(
                                ps[:, :], wnat[:, ft, kt * 128:(kt + 1) * 128],
                                ident[:, :],
                            )
                            if name == "wk":
                                # permute output features to g-major:
                                # f = 16*dl + g + 128*ft  ->  f' = 64*g + 8*ft + dl
                                src = ap(ps, 0, [[16, 8], [1, 16]])       # (dl, g)
                                dst = ap(wt, kt * H + 8 * ft,
                                         [[1, 8], [64, 16]])              # (dl, g)
                                nc.scalar.copy(out=dst, in_=src)
                            else:
                                nc.scalar.copy(
                                    out=wt[:, kt, ft * 128:(ft + 1) * 128],
                                    in_=ps[:, :],
                                )

            with tc.tile_pool(name="io", bufs=2) as io_pool, \
                 tc.tile_pool(name="acts", bufs=2) as act_pool, \
                 tc.tile_pool(name="prod", bufs=2) as prod_pool, \
                 tc.tile_pool(name="small", bufs=2) as small_pool, \
                 tc.tile_pool(name="tps", bufs=4, space="PSUM") as t_psum, \
                 tc.tile_pool(name="mmps", bufs=4, space="PSUM") as mm_psum:

                for it in range(ntiles):
                    r0 = it * 128

                    h_sb = io_pool.tile([128, H], f32, tag="h")
                    nc.sync.dma_start(out=h_sb, in_=h_d[r0:r0 + 128, :])

                    # hT[p, c, j] = h[r0 + j, c*128 + p]
                    hT = act_pool.tile([128, KT, 128], f32r, tag="hT")
                    for c in range(KT):
                        ps = t_psum.tile([128, 128], f32, tag="tp")
                        nc.tensor.transpose(
                            ps[:, :], h_sb[:, c * 128:(c + 1) * 128], ident[:, :]
                        )
                        nc.scalar.copy(out=hT[:, c, :], in_=ps[:, :])

                    # Projections -> bf16 activations.
                    # qb: (hh, d) row-major;  kb: (g, d) [via permuted wk];
                    # vb: (d, g) row-major.
                    projs = {}
                    for name, pname in (("wq", "qb"), ("wk", "kb"), ("wv", "vb")):
                        dst = act_pool.tile([128, H], bf16, tag=pname)
                        projs[pname] = dst
                        for half in range(2):
                            ps = mm_psum.tile([128, 512], f32, tag="mm")
                            for kt in range(KT):
                                nc.tensor.matmul(
                                    ps[:, :],
                                    hT[:, kt, :],
                                    wts[name][:, kt, half * 512:(half + 1) * 512],
                                    start=(kt == 0),
                                    stop=(kt == KT - 1),
                                )
                            nc.scalar.copy(
                                out=dst[:, half * 512:(half + 1) * 512], in_=ps[:, :]
                            )
                    qb, kb, vb = projs["qb"], projs["kb"], projs["vb"]

                    # comp[hh, g] = sum_d qb[hh*64+d] * kb[g*64+d]
                    comp = small_pool.tile([128, NH, NH], f32, tag="comp")
                    for qq in range(4):  # 4 heads per pass
                        p1 = prod_pool.tile([128, 4, NH, HD], bf16, tag="prod")
                        in0 = ap(qb, qq * 4 * HD, [[HD, 4], [0, NH], [1, HD]])
                        in1 = ap(kb, 0, [[0, 4], [HD, NH], [1, HD]])
                        nc.vector.tensor_tensor(p1[...], in0, in1, MULT)
                        nc.vector.tensor_reduce(
                            comp[:, qq * 4:(qq + 1) * 4, :], p1[...], AXX, ADD
                        )

                    # softmax over g (scale by 1/128 inside exp)
                    e = small_pool.tile([128, NH, NH], f32, tag="e")
                    nc.scalar.activation(
                        e[...], comp[...], mybir.ActivationFunctionType.Exp,
                        scale=1.0 / 128.0,
                    )
                    s = small_pool.tile([128, NH], f32, tag="s")
                    nc.vector.tensor_reduce(s[...], e[...], AXX, ADD)
                    r = small_pool.tile([128, NH], f32, tag="r")
                    nc.vector.reciprocal(r[...], s[...])
                    scores = small_pool.tile([128, NH, NH], bf16, tag="sc")
                    nc.vector.tensor_tensor(
                        scores[...], e[...], ap(r, 0, [[1, NH], [0, NH]]), MULT
                    )

                    # out[l, d] = sum_g scores[l, g] * vb[d*16+g]
                    # OUT flat index = 16*d + l
                    OUT = act_pool.tile([128, H], f32, tag="out")
                    for dq in range(4):  # 16 d per pass
                        p2 = prod_pool.tile([128, NH, NH, NH], bf16, tag="prod")
                        in0 = ap(scores, 0, [[0, NH], [NH, NH], [1, NH]])
                        in1 = ap(vb, dq * NH * NH, [[NH, NH], [0, NH], [1, NH]])
                        nc.vector.tensor_tensor(p2[...], in0, in1, MULT)
                        nc.vector.tensor_reduce(
                            ap(OUT, dq * 256, [[1, 256]]).rearrange(
                                "p (a b) -> p a b", a=NH
                            ),
                            p2[...], AXX, ADD,
                        )

                    # outT[p, c, j] = OUT[j, c*128 + p]
                    outT = act_pool.tile([128, KT, 128], f32r, tag="outT")
                    for c in range(KT):
                        ps = t_psum.tile([128, 128], f32, tag="tp")
                        nc.tensor.transpose(
                            ps[:, :], OUT[:, c * 128:(c + 1) * 128], ident[:, :]
                        )
                        nc.scalar.copy(out=outT[:, c, :], in_=ps[:, :])

                    final = io_pool.tile([128, H], f32, tag="final")
                    for half in range(2):
                        ps = mm_psum.tile([128, 512], f32, tag="mm")
                        for kt in range(KT):
                            nc.tensor.matmul(
                                ps[:, :],
                                outT[:, kt, :],
                                wts["wfc"][:, kt, half * 512:(half + 1) * 512],
                                start=(kt == 0),
                                stop=(kt == KT - 1),
                            )
                        nc.scalar.copy(
                            out=final[:, half * 512:(half + 1) * 512], in_=ps[:, :]
                        )
                    nc.sync.dma_start(out=out_d[r0:r0 + 128, :], in_=final)

    nc.compile()
    _BUILD_CACHE[n_rows] = nc
    return nc


def kernel(h, Wq, Wk, Wv, Wfc):
    from concourse import bass_utils

    h = np.ascontiguousarray(np.asarray(h, dtype=np.float32))
    ws = {
        "wq": np.ascontiguousarray(np.asarray(Wq, dtype=np.float32)),
        "wk": np.ascontiguousarray(np.asarray(Wk, dtype=np.float32)),
        "wv": np.ascontiguousarray(np.asarray(Wv, dtype=np.float32)),
        "wfc": np.ascontiguousarray(np.asarray(Wfc, dtype=np.float32)),
    }
    nc = _build(NPC)
    in_maps = [
        {"h": h[i * NPC:(i + 1) * NPC], **ws} for i in range(NCORES)
    ]
    res = bass_utils.run_bass_kernel_spmd(nc, in_maps, core_ids=list(range(NCORES)))
    return np.concatenate(
        [res.results[i]["out"] for i in range(NCORES)], axis=0
    ).astype(np.float32)


# revision 9
# speedup vs baseline: 11414.1580x; 11414.1580x over previous
"""Trainium2 Bass kernel for per-node multi-head attention.

Computation (per node n, fully independent across nodes):
    Q = h @ Wq.T  viewed (nh, hd)        [row-major reshape]
    K = h @ Wk.T  viewed (hd, nh)
    V = h @ Wv.T  viewed (hd, nh)
    comp[hh, g] = sum_d Q[hh, d] K[d, g] / 128
    scores = softmax(comp, axis=-1)
    out[l, d]  = sum_g scores[l, g] V[d, g]
    final = flat(out.T) @ Wfc.T

Sharding: data-parallel over the node dim N across 8 NeuronCores; weights
replicated; no collectives.

Per-core mapping:
  - TensorE: transpose h blocks (128x128), the 3 projections + final FC as
    float32r matmuls (full-rate fp32), transpose of the attention output.
  - VectorE: batched per-node einsums as broadcast-AP elementwise products
    (bf16) + segmented tensor_reduce, plus softmax normalization.
  - ScalarE: exp, PSUM->SBUF copies (with dtype casts).
  - Wk's rows are permuted during on-device weight prep so the K projection
    lands g-major (f' = g*64 + d), which makes the comp product APs unit-
    stride in the innermost dim.
"""

import numpy as np

N_FULL = 65536
H = 1024
NCORES = 8
NPC = N_FULL // NCORES  # rows per core
NH = 16                 # heads
HD = 64                 # head dim
KT = H // 128           # k tiles per contraction (8)

_BUILD_CACHE = {}


def _build(n_rows, ablate=()):
    """ablate: subset of {'attn', 'mm', 'tpose', 'copies'} — skip those parts
    (wrong results; used only for TimelineSim attribution experiments)."""
    key = (n_rows, tuple(sorted(ablate)))
    if key in _BUILD_CACHE:
        return _BUILD_CACHE[key]

    import concourse.bass as bass
    import concourse.mybir as mybir
    import concourse.tile as tile
    from concourse import bacc
    from concourse.masks import make_identity

    f32 = mybir.dt.float32
    f32r = mybir.dt.float32r
    bf16 = mybir.dt.bfloat16
    MULT = mybir.AluOpType.mult
    ADD = mybir.AluOpType.add
    AXX = mybir.AxisListType.X

    nc = bacc.Bacc("TRN2", target_bir_lowering=False, debug=False)

    h_d = nc.dram_tensor("h", [n_rows, H], f32, kind="ExternalInput").ap()
    w_d = {
        name: nc.dram_tensor(name, [H, H], f32, kind="ExternalInput").ap()
        for name in ("wq", "wk", "wv", "wfc")
    }
    out_d = nc.dram_tensor("out", [n_rows, H], f32, kind="ExternalOutput").ap()

    ntiles = n_rows // 128

    def ap(base, offset_elems, dims):
        """Manual AP: dims are [step, count] FREE dims; partition from base."""
        b = base if isinstance(base, bass.AP) else base[...]
        return bass.AP(
            tensor=b.tensor,
            offset=b.offset + offset_elems,
            ap=[list(b.ap[0])] + [list(d) for d in dims],
        )

    with tile.TileContext(nc) as tc:
        with tc.tile_pool(name="const", bufs=1) as const_pool:
            ident = const_pool.tile([128, 128], f32)
            make_identity(nc, ident)

            # Transposed weights, SBUF-resident for the whole kernel.
            # wt[p, kt, f] = W[f, kt*128 + p]   (for wk: f is permuted g-major)
            wts = {}
            with tc.tile_pool(name="wprep", bufs=2) as wnat_pool, \
                 tc.tile_pool(name="wtps", bufs=4, space="PSUM") as wt_psum:
                for name in ("wq", "wk", "wv", "wfc"):
                    wt = const_pool.tile([128, KT, H], f32r, tag=f"wt_{name}")
                    wts[name] = wt
                    wnat = wnat_pool.tile([128, KT, H], f32, tag="wnat")
                    nc.sync.dma_start(
                        out=wnat,
                        in_=w_d[name].rearrange("(ft p) c -> p ft c", p=128),
                    )
                    for ft in range(KT):
                        for kt in range(KT):
                            ps = wt_psum.tile([128, 128], f32, tag="wt_ps")
                            nc.tensor.transpose(
                                ps[:, :], wnat[:, ft, kt * 128:(kt + 1) * 128],
                                ident[:, :],
                            )
                            if name == "wk":
                                # permute output features to g-major:
                                # f = 16*dl + g + 128*ft  ->  f' = 64*g + 8*ft + dl
                                src = ap(ps, 0, [[16, 8], [1, 16]])       # (dl, g)
                                dst = ap(wt, kt * H + 8 * ft,
                                         [[1, 8], [64, 16]])              # (dl, g)
                                nc.scalar.copy(out=dst, in_=src)
                            else:
                                nc.scalar.copy(
                                    out=wt[:, kt, ft * 128:(ft + 1) * 128],
                                    in_=ps[:, :],
                                )

            with tc.tile_pool(name="io", bufs=2) as io_pool, \
                 tc.tile_pool(name="acts", bufs=2) as act_pool, \
                 tc.tile_pool(name="prod", bufs=2) as prod_pool, \
                 tc.tile_pool(name="small", bufs=2) as small_pool, \
                 tc.tile_pool(name="tps", bufs=4, space="PSUM") as t_psum, \
                 tc.tile_pool(name="mmps", bufs=4, space="PSUM") as mm_psum:

                for it in range(ntiles):
                    r0 = it * 128

                    h_sb = io_pool.tile([128, H], f32, tag="h")
                    nc.sync.dma_start(out=h_sb, in_=h_d[r0:r0 + 128, :])

                    # hT[p, c, j] = h[r0 + j, c*128 + p]
                    hT = act_pool.tile([128, KT, 128], f32r, tag="hT")
                    for c in range(KT if "tpose" not in ablate else 0):
                        ps = t_psum.tile([128, 128], f32, tag="tp")
                        nc.tensor.transpose(
                            ps[:, :], h_sb[:, c * 128:(c + 1) * 128], ident[:, :]
                        )
                        nc.scalar.copy(out=hT[:, c, :], in_=ps[:, :])

                    # Projections -> bf16 activations.
                    # qb: (hh, d) row-major;  kb: (g, d) [via permuted wk];
                    # vb: (d, g) row-major.
                    projs = {}
                    for name, pname in (("wq", "qb"), ("wk", "kb"), ("wv", "vb")):
                        dst = act_pool.tile([128, H], bf16, tag=pname)
                        projs[pname] = dst
                        for half in range(2):
                            ps = mm_psum.tile([128, 512], f32, tag="mm")
                            for kt in range(KT if "mm" not in ablate else 0):
                                nc.tensor.matmul(
                                    ps[:, :],
                                    hT[:, kt, :],
                                    wts[name][:, kt, half * 512:(half + 1) * 512],
                                    start=(kt == 0),
                                    stop=(kt == KT - 1),
                                )
                            nc.scalar.copy(
                                out=dst[:, half * 512:(half + 1) * 512], in_=ps[:, :]
                            )
                    qb, kb, vb = projs["qb"], projs["kb"], projs["vb"]

                    # comp[hh, g] = sum_d qb[hh*64+d] * kb[g*64+d]
                    comp = small_pool.tile([128, NH, NH], f32, tag="comp")
                    for qq in range(4 if "attn" not in ablate else 0):  # 4 heads per pass
                        p1 = prod_pool.tile([128, 4, NH, HD], bf16, tag="prod")
                        in0 = ap(qb, qq * 4 * HD, [[HD, 4], [0, NH], [1, HD]])
                        in1 = ap(kb, 0, [[0, 4], [HD, NH], [1, HD]])
                        nc.vector.tensor_tensor(p1[...], in0, in1, MULT)
                        # bf16 add-tree (2x mode) halves the 1x reduce cost:
                        # d: 64 -> 32 -> 16, then a short fp32 reduce over 16
                        tr = prod_pool.tile([128, 4096], bf16, tag="prod")
                        nc.vector.tensor_tensor(
                            ap(tr, 0, [[32, 64], [1, 32]]),
                            ap(p1, 0, [[64, 64], [1, 32]]),
                            ap(p1, 32, [[64, 64], [1, 32]]), ADD)
                        nc.vector.tensor_tensor(
                            ap(tr, 2048, [[16, 64], [1, 16]]),
                            ap(tr, 0, [[32, 64], [1, 16]]),
                            ap(tr, 16, [[32, 64], [1, 16]]), ADD)
                        nc.vector.tensor_reduce(
                            comp[:, qq * 4:(qq + 1) * 4, :],
                            ap(tr, 2048, [[16, 64], [1, 16]]), AXX, ADD
                        )

                    # softmax over g (scale by 1/128 inside exp)
                    e = small_pool.tile([128, NH, NH], f32, tag="e")
                    nc.scalar.activation(
                        e[...], comp[...], mybir.ActivationFunctionType.Exp,
                        scale=1.0 / 128.0,
                    )
                    s = small_pool.tile([128, NH], f32, tag="s")
                    nc.vector.tensor_reduce(s[...], e[...], AXX, ADD)
                    r = small_pool.tile([128, NH], f32, tag="r")
                    nc.vector.reciprocal(r[...], s[...])
                    scores = small_pool.tile([128, NH, NH], bf16, tag="sc")
                    nc.vector.tensor_tensor(
                        scores[...], e[...], ap(r, 0, [[1, NH], [0, NH]]), MULT
                    )

                    # out[l, d] = sum_g scores[l, g] * vb[d*16+g]
                    # OUT flat index = 16*d + l
                    OUT = act_pool.tile([128, H], f32, tag="out")
                    for dq in range(4 if "attn" not in ablate else 0):  # 16 d per pass
                        p2 = prod_pool.tile([128, NH, NH, NH], bf16, tag="prod")
                        in0 = ap(scores, 0, [[0, NH], [NH, NH], [1, NH]])
                        in1 = ap(vb, dq * NH * NH, [[NH, NH], [0, NH], [1, NH]])
                        nc.vector.tensor_tensor(p2[...], in0, in1, MULT)
                        # g: 16 -> 8 -> 4, then fp32 reduce over 4
                        tr = prod_pool.tile([128, 4096], bf16, tag="prod")
                        nc.vector.tensor_tensor(
                            ap(tr, 0, [[8, 256], [1, 8]]),
                            ap(p2, 0, [[16, 256], [1, 8]]),
                            ap(p2, 8, [[16, 256], [1, 8]]), ADD)
                        nc.vector.tensor_tensor(
                            ap(tr, 2048, [[4, 256], [1, 4]]),
                            ap(tr, 0, [[8, 256], [1, 4]]),
                            ap(tr, 4, [[8, 256], [1, 4]]), ADD)
                        nc.vector.tensor_reduce(
                            ap(OUT, dq * 256, [[1, 256]]).rearrange(
                                "p (a b) -> p a b", a=NH
                            ),
                            ap(tr, 2048, [[4, 256], [1, 4]]), AXX, ADD,
                        )

                    # outT[p, c, j] = OUT[j, c*128 + p]
                    outT = act_pool.tile([128, KT, 128], f32r, tag="outT")
                    for c in range(KT if "tpose" not in ablate else 0):
                        ps = t_psum.tile([128, 128], f32, tag="tp")
                        nc.tensor.transpose(
                            ps[:, :], OUT[:, c * 128:(c + 1) * 128], ident[:, :]
                        )
                        nc.scalar.copy(out=outT[:, c, :], in_=ps[:, :])

                    final = io_pool.tile([128, H], f32, tag="final")
                    for half in range(2):
                        ps = mm_psum.tile([128, 512], f32, tag="mm")
                        for kt in range(KT if "mm" not in ablate else 0):
                            nc.tensor.matmul(
                                ps[:, :],
                                outT[:, kt, :],
                                wts["wfc"][:, kt, half * 512:(half + 1) * 512],
                                start=(kt == 0),
                                stop=(kt == KT - 1),
                            )
                        nc.scalar.copy(
                            out=final[:, half * 512:(half + 1) * 512], in_=ps[:, :]
                        )
                    nc.sync.dma_start(out=out_d[r0:r0 + 128, :], in_=final)

    nc.compile()
    _BUILD_CACHE[key] = nc
    return nc


def kernel(h, Wq, Wk, Wv, Wfc):
    from concourse import bass_utils

    h = np.ascontiguousarray(np.asarray(h, dtype=np.float32))
    ws = {
        "wq": np.ascontiguousarray(np.asarray(Wq, dtype=np.float32)),
        "wk": np.ascontiguousarray(np.asarray(Wk, dtype=np.float32)),
        "wv": np.ascontiguousarray(np.asarray(Wv, dtype=np.float32)),
        "wfc": np.ascontiguousarray(np.asarray(Wfc, dtype=np.float32)),
    }
    nc = _build(NPC)
    in_maps = [
        {"h": h[i * NPC:(i + 1) * NPC], **ws} for i in range(NCORES)
    ]
    res = bass_utils.run_bass_kernel_spmd(nc, in_maps, core_ids=list(range(NCORES)))
    return np.concatenate(
        [res.results[i]["out"] for i in range(NCORES)], axis=0
    ).astype(np.float32)
